# revision 72
# baseline (speedup 1.0000x reference)
"""Distributed Trainium2 attention-head kernel (softmax over the QUERY axis).

Strategy (8 NeuronCores, SPMD, KEY-dim sharding, zero cross-core comm):
  The softmax normalizes over the query axis (axis 0 of scores). Sharding
  the KEY dim keeps every softmax column fully local to one core: core c
  holds keys/values rows [c*1024:(c+1)*1024] and the FULL queries. It
  computes scoresT[j, i] for its 1024 keys x all 8192 queries, local
  per-key softmax stats, and a full-shape partial output
  outT_c = (attn_slab)^T-contracted with v_slab. The HOST sums the 8
  partials. No device-to-device traffic -> immune to the multi-ms
  execution-start stagger across the 8 PJRT devices (which dominated the
  previous all-to-all design at ~11.7ms).

  Numerics (validated vs reference in fp64/numpy: rel err 2.9e-3):
  - Host pre-splits activations/weights into bf16 hi/lo pairs.
  - 3-term bf16 projections (wh*xh + wh*xl + wl*xh) in fp32 PSUM.
  - Projected q/k re-split hi/lo on device via PSUM bf16-rounding trick.
  - Scores: 2-pass stacked bf16 matmuls: kT_A=[kh;kl], kT_B=[kl;kh]
    against qS=[qh;ql] -> exact (kh+kl)(qh+ql) in fp32 PSUM.
  - Softmax over queries with per-512-chunk max; the exp(m_chunk - m_j)/S_j
    rescale folds into per-chunk vt tiles used as attn@v lhsT.
"""

import numpy as np

C = 8
SEQ = 8192
D = 1024
QK = 64
VD = 64


def build_nc(seq=SEQ, d=D, warmup=True, debug_taps=False):
    import concourse.bacc as bacc
    import concourse.mybir as mybir

    f32 = mybir.dt.float32
    bf16 = mybir.dt.bfloat16
    AX = mybir.AxisListType.X
    ALU = mybir.AluOpType
    ACTF = mybir.ActivationFunctionType

    NDT = d // 128            # 8 d_model tiles
    KSL = seq // C            # 1024 keys per core
    KT = KSL // 128           # 8 key tiles
    NSEC = 8                  # q-projection sections
    SECW = seq // NSEC        # 1024 query cols per section
    NCHK = seq // 512         # 16 score chunks per key tile
    NIDX = KT * NCHK          # 128 (j, c) chunk pairs
    QIN_W = NDT * seq         # 65536 q activation cols
    KIN_W = NDT * KSL         # 8192 k/v activation cols
    PCW = 2048                # q piece width (2 dd per piece, swizzled DRAM)
    NPPS = NDT * SECW * 2 // PCW  # 8 pieces per section (4 per hl)
    NPIECE = NSEC * NPPS      # 64 streamed q pieces

    nc = bacc.Bacc(target_bir_lowering=False, debug=False)

    def din(name, w):
        return nc.declare_dram_parameter(name, [128, w], bf16, isOutput=False)

    qhT_d, qlT_d = din("qhT", QIN_W), din("qlT", QIN_W)
    khT_d, klT_d = din("khT", KIN_W), din("klT", KIN_W)
    vT_d = din("vT", KIN_W)
    wqs_d = din("wqs", NDT * 2 * QK)   # stacked [wqh | wql], 128 cols per dd
    wkh_d, wkl_d = din("wkh", NDT * QK), din("wkl", NDT * QK)
    wv_d = din("wv", NDT * VD)
    out_d = nc.declare_dram_parameter("out", [VD, seq], f32, isOutput=True)
    dbg = {}
    if debug_taps:
        def dout(name, p, w, dt):
            dbg[name] = nc.declare_dram_parameter(name, [p, w], dt, isOutput=True)

        dout("d_qU", 128, seq, bf16)
        dout("d_qV", 128, seq, bf16)
        dout("d_kTA", 128, seq // C, bf16)
        dout("d_kTB", 128, seq // C, bf16)
        dout("d_vsb", 128, (seq // C // 128) * VD, bf16)
        dout("d_negm", 128, (seq // C // 128) * (seq // 512), f32)
        dout("d_S", 128, (seq // C // 128) * (seq // 512), f32)
        dout("d_emat", 128, (seq // C // 128) * (seq // 512), f32)
        dout("d_mt1", 128, (seq // C // 128) * 8, f32)
        dout("d_negM", 128, seq // C // 128, f32)
        dout("d_dmat", 128, (seq // C // 128) * (seq // 512), f32)
        dout("d_Sg", 128, seq // C // 128, f32)
        dout("d_gg", 128, (seq // C // 128) * (seq // 512), f32)
        dout("d_vt", 128, (seq // C // 128) * (seq // 512) * VD, bf16)
        dout("d_P", 128, (seq // C // 128) * seq, bf16)

    from contextlib import ExitStack

    with ExitStack() as ctx:
        block = ctx.enter_context(nc.Block())
        sem = lambda n: ctx.enter_context(nc.semaphore(n))
        sb = lambda n, shape, dt: ctx.enter_context(nc.sbuf_tensor(n, shape, dt))
        ps = lambda n, shape: ctx.enter_context(nc.psum_tensor(n, shape, f32))

        s_in_k = sem("s_in_k")
        s_in_v = sem("s_in_v")
        s_in_q = sem("s_in_q")
        # per-slot DMA-completion sems: a single counting sem is racy when
        # two DMAs are in flight (16 per-engine incs interleave across DMAs)
        NSLOT = 7
        s_qd = [sem(f"s_qd{i}") for i in range(NSLOT)]
        s_odp = [sem("s_od0"), sem("s_od1")]
        s_qcons = sem("s_qcons")
        s_qsp = sem("s_qsp")
        s_kproj = sem("s_kproj")
        s_ksp = sem("s_ksp")
        s_vproj = sem("s_vproj")
        s_vcp = sem("s_vcp")
        s_sc = sem("s_sc")
        s_mx = sem("s_mx")
        s_exp = sem("s_exp")
        s_dm = sem("s_dm")
        s_em = sem("s_em")
        s_mM = sem("s_mM")
        s_rs = sem("s_rs")
        s_gq = sem("s_gq")
        s_vch = sem("s_vch")
        s_vt = sem("s_vt")
        s_vta = sem("s_vta")
        s_qcp = sem("s_qcp")
        s_attn = sem("s_attn")
        s_ocp = sem("s_ocp")
        s_od = sem("s_od")

        # ---- SBUF ----
        p_sb = sb("p_arena", [128, KT * seq], bf16)     # P; aliases k/v acts
        khT = p_sb[:, 0 * KIN_W : 1 * KIN_W]
        klT = p_sb[:, 1 * KIN_W : 2 * KIN_W]
        vT = p_sb[:, 2 * KIN_W : 3 * KIN_W]
        qU = sb("qU", [128, seq], bf16)                 # [ah(0:64); bh(64:128)]
        qV = sb("qV", [128, seq], bf16)                 # [al(0:64); bl(64:128)]
        # streaming slots share storage with vt_all: slots die after the last
        # qproj consumption, vt_all is written at combine (strictly later)
        qvt = sb("qvt_arena", [128, NSLOT * PCW], bf16)
        qslot = qvt[:, :]
        kT_A = sb("kT_A", [128, KSL], bf16)             # [kh; kh]
        kT_B = sb("kT_B", [128, KSL], bf16)             # [kl; kl]
        wqs = sb("wqs_s", [128, NDT * 2 * QK], bf16)
        wkh = sb("wkh_s", [128, NDT * QK], bf16)
        wkl = sb("wkl_s", [128, NDT * QK], bf16)
        wv = sb("wv_s", [128, NDT * VD], bf16)
        v_sb = sb("v_sb", [128, KT * VD], bf16)         # projected v slab
        vt_all = qvt[:, 0 : NIDX * VD]                  # per-(j,c) vt tiles
        negm = sb("negm", [128, NIDX], f32)             # -max per (j,c)
        S_ = sb("S_", [128, NIDX], f32)                 # exp-sum per (j,c)
        mt1 = sb("mt1", [128, KT * 8], f32)
        mt2 = sb("mt2", [128, KT * 4], f32)
        mt3 = sb("mt3", [128, KT * 2], f32)
        negM = sb("negM", [128, KT], f32)               # -max per j
        dmat = sb("dmat", [128, NIDX], f32)             # m_j - m_jc
        emat = sb("emat", [128, NIDX], f32)             # exp(m_jc - m_j)
        Sw = sb("Sw", [128, NIDX], f32)
        Sg = sb("Sg", [128, KT], f32)
        rS = sb("rS", [128, KT], f32)
        gg = sb("gg", [128, NIDX], f32)                 # e/S fold factors
        out_sb = sb("out_sb", [64, 1024], f32)

        # ---- PSUM: 2-bank ping-pong pair for q-proj + 4-bank scratch ----
        ps_qA = ps("ps_qA", [128, SECW])      # 2 banks
        ps_qB = ps("ps_qB", [128, SECW])      # 2 banks
        ps_sc = ps("ps_sc", [128, 2048])      # 4 banks: k/v proj, scores, attn
        kps = ps_sc[:, 0:KSL]                 # k-projection [128, 1024]
        vps = ps_sc[:, KSL : KSL + KT * VD]   # v-projection [128, 512]
        sbank = [
            ps_sc[:, 0:512],
            ps_sc[:, 512:1024],
            ps_sc[:, 1024:1536],
            ps_sc[:, 1536:2048],
        ]
        NBANK = 4
        # all at partition base 0: DVE copies to out_sb must stay lane-aligned
        atile = [
            ps_qA[0:64, :],
            ps_qB[0:64, :],
            ps_sc[0:64, 0:1024],
            ps_sc[0:64, 1024:2048],
        ]

        def jv3(t2d, j, c):
            return t2d[:, 0 : j * c].rearrange("p (j c) -> p j c", j=j, c=c)

        # ---------------- SYNC: all input DMAs + output ----------------
        @block.sync
        def _(s):
            def dma(dst, src, sem_):
                s.dma_start(out=dst, in_=src).then_inc(sem_, 16)

            dma(wkh[:, :], wkh_d[:, :], s_in_k)
            dma(wkl[:, :], wkl_d[:, :], s_in_k)
            dma(khT, khT_d[:, :], s_in_k)
            dma(klT, klT_d[:, :], s_in_k)
            dma(wv[:, :], wv_d[:, :], s_in_v)
            dma(vT, vT_d[:, :], s_in_v)
            dma(wqs[:, :], wqs_d[:, :], s_in_q)
            # q activation stream. DRAM is host-swizzled: col layout is
            # sec-major (sec, dd, i) so a piece = (sec, hl, dd-pair) is one
            # contiguous 512 KB read.
            for p in range(NPIECE):
                sec, rem = divmod(p, NPPS)
                hl, pr = divmod(rem, NPPS // 2)
                if p >= NSLOT:
                    s.wait_ge(s_qcons, p - NSLOT + 1)
                src_d = qhT_d if hl == 0 else qlT_d
                s.dma_start(
                    out=qslot[:, (p % NSLOT) * PCW : (p % NSLOT + 1) * PCW],
                    in_=src_d[:, sec * NDT * SECW + pr * PCW :
                              sec * NDT * SECW + (pr + 1) * PCW],
                ).then_inc(s_qd[p % NSLOT], 16)
            # output tiles (single out_sb slot: one DMA in flight at a time)
            for t in range(8):
                s.wait_ge(s_ocp, t + 1)
                s.dma_start(
                    out=out_d[:, t * 1024 : (t + 1) * 1024],
                    in_=out_sb[:, :],
                ).then_inc(s_odp[0], 16)
            s.wait_ge(s_odp[0], 8 * 16)
            if debug_taps:
                n_dbg = 0
                for name, src in (
                    ("d_qU", qU[:, :]),
                    ("d_qV", qV[:, :]),
                    ("d_kTA", kT_A[:, :]),
                    ("d_kTB", kT_B[:, :]),
                    ("d_vsb", v_sb[:, :]),
                    ("d_negm", negm[:, :]),
                    ("d_S", S_[:, :]),
                    ("d_emat", emat[:, :]),
                    ("d_mt1", mt1[:, :]),
                    ("d_negM", negM[:, :]),
                    ("d_dmat", dmat[:, :]),
                    ("d_Sg", Sg[:, :]),
                    ("d_gg", gg[:, :]),
                    ("d_vt", vt_all[:, :]),
                    ("d_P", p_sb[:, :]),
                ):
                    s.dma_start(out=dbg[name][:, :], in_=src).then_inc(s_odp[0], 16)
                    n_dbg += 1
                s.wait_ge(s_odp[0], (8 + n_dbg) * 16)

        # ---------------- SCALAR (ACT): q-split copies + exp + vt ----------
        @block.scalar
        def _(sc):
            def qcopy(sec):
                # bf16 rounding copies of the projection PSUM; DVE derives
                # the residuals. Runs on ACT so DVE's max->exp chain never
                # waits behind split work.
                sc.wait_ge(s_qcons, NPPS * (sec + 1))
                buf = (ps_qA, ps_qB)[sec % 2]
                sl = slice(sec * SECW, (sec + 1) * SECW)
                sc.activation(qU[0:64, sl], buf[0:64, :], ACTF.Copy)
                sc.activation(qU[64:128, sl], buf[64:128, :], ACTF.Copy)\
                    .then_inc(s_qcp, 1)

            def exps(sec, lo, hi):
                for i2 in range(lo, hi):
                    idx = sec * 2 * KT + i2
                    c2, j = divmod(i2, KT)
                    c = sec * 2 + c2
                    col = j * NCHK + c
                    sc.wait_ge(s_mx, idx + 1)
                    sc.activation(
                        p_sb[:, j * seq + c * 512 : j * seq + (c + 1) * 512],
                        sbank[idx % NBANK],
                        ACTF.Exp,
                        bias=negm[:, col : col + 1],
                        scale=1.0,
                        accum_out=S_[:, col : col + 1],
                    ).then_inc(s_exp, 1)

            # qcopy(sec+2) tucked between exps so the scores bank rotation
            # never waits behind split-copy work
            qcopy(0)
            qcopy(1)
            for sec in range(NSEC):
                exps(sec, 0, 12)
                if sec + 2 < NSEC:
                    qcopy(sec + 2)
                exps(sec, 12, 16)

            # combine: emat_j = exp(-negm_jc + negM_j) = exp(m_jc - m_j)
            sc.wait_ge(s_mM, 1)
            for j in range(KT):
                ins = sc.activation(
                    emat[:, j * NCHK : (j + 1) * NCHK],
                    negm[:, j * NCHK : (j + 1) * NCHK],
                    ACTF.Exp,
                    scale=-1.0,
                    bias=negM[:, j : j + 1],
                )
            ins.then_inc(s_em, 1)

            # odd-chunk vt tiles: vt = v * gg via Copy-with-scale
            sc.wait_ge(s_gq, 1)
            for c in range(1, NCHK, 2):
                for j in range(KT):
                    idx = j * NCHK + c
                    ins = sc.activation(
                        vt_all[:, idx * VD : (idx + 1) * VD],
                        v_sb[:, j * VD : (j + 1) * VD],
                        ACTF.Copy,
                        scale=gg[:, idx : idx + 1],
                    )
                ins.then_inc(s_vta, 1)

        # ---------------- TENSOR ----------------
        @block.tensor
        def _(t):
            # HAM warm-up junk matmuls while k/v DMAs stream.
            for w in range(58 if warmup else 0):
                t.matmul(
                    kps[0:64, 0:512],
                    p_sb[:, 0:64],
                    p_sb[:, 64 : 64 + 512],
                    start=(w == 0),
                    stop=False,
                )

            # k projection -> kps, duplicated on both partition halves.
            t.wait_ge(s_in_k, 64)
            for pos in (0, 64):
                for dd in range(NDT):
                    for ti, (W, X) in enumerate(
                        ((wkh, khT), (wkh, klT), (wkl, khT))
                    ):
                        for n in range(2):
                            mm = t.matmul(
                                kps[pos : pos + 64, n * 512 : (n + 1) * 512],
                                W[:, dd * QK : (dd + 1) * QK],
                                X[:, dd * KSL + n * 512 : dd * KSL + (n + 1) * 512],
                                start=(dd == 0 and ti == 0),
                                stop=(dd == NDT - 1 and ti == 2),
                                tile_position=(0, pos),
                            )
            mm.then_inc(s_kproj, 1)

            # v projection -> vps
            t.wait_ge(s_in_v, 32)
            for tau in range(KT):
                for dd in range(NDT):
                    mm = t.matmul(
                        vps[:, tau * VD : (tau + 1) * VD],
                        vT[:, dd * KSL + tau * 128 : dd * KSL + tau * 128 + 128],
                        wv[:, dd * VD : (dd + 1) * VD],
                        start=(dd == 0),
                        stop=(dd == NDT - 1),
                    )
            mm.then_inc(s_vproj, 1)

            # q projection in 8 sections of 1024 (stacked [wqh|wql] weights:
            # psum[0:64]=a, psum[64:128]=b), scores lag one section behind
            t.wait_ge(s_in_q, 16)

            def qproj(sec):
                if sec >= 2:
                    t.wait_ge(s_qsp, sec - 1)
                buf = (ps_qA, ps_qB)[sec % 2]
                for hl in range(2):
                    for dd in range(NDT):
                        pp = sec * NPPS + hl * (NPPS // 2) + dd // 2
                        half = (dd % 2) * SECW
                        if dd % 2 == 0:
                            t.wait_ge(s_qd[pp % NSLOT], 16 * (pp // NSLOT + 1))
                        X = qslot[:, (pp % NSLOT) * PCW + half :
                                  (pp % NSLOT) * PCW + half + SECW]
                        for n in range(2):
                            mm = t.matmul(
                                buf[:, n * 512 : (n + 1) * 512],
                                wqs[:, dd * 128 : (dd + 1) * 128],
                                X[:, n * 512 : (n + 1) * 512],
                                start=(hl == 0 and dd == 0),
                                stop=(hl == 1 and dd == NDT - 1),
                            )
                        if dd % 2 == 1:
                            mm.then_inc(s_qcons, 1)

            def scores(sec):
                t.wait_ge(s_qsp, sec + 1)
                for c2 in range(2):
                    c = sec * 2 + c2
                    for j in range(KT):
                        idx = sec * 2 * KT + c2 * KT + j
                        if idx >= NBANK:
                            t.wait_ge(s_exp, idx - NBANK + 1)
                        bank = sbank[idx % NBANK]
                        t.matmul(
                            bank,
                            kT_A[:, j * 128 : (j + 1) * 128],
                            qU[:, c * 512 : (c + 1) * 512],
                            start=True,
                            stop=False,
                        )
                        t.matmul(
                            bank,
                            kT_A[:, j * 128 : (j + 1) * 128],
                            qV[:, c * 512 : (c + 1) * 512],
                            start=False,
                            stop=False,
                        )
                        mm = t.matmul(
                            bank,
                            kT_B[:, j * 128 : (j + 1) * 128],
                            qU[:, c * 512 : (c + 1) * 512],
                            start=False,
                            stop=True,
                        )
                        mm.then_inc(s_sc, 1)

            for sec in range(NSEC):
                qproj(sec)
                if sec == 0:
                    # scores write ps_sc: k/v proj results must be consumed
                    t.wait_ge(s_ksp, 1)
                    t.wait_ge(s_vcp, 1)
                else:
                    scores(sec - 1)
            scores(NSEC - 1)

            # attn@v: 8 output tiles of [64, 1024], accumulate over j;
            # per-tile gating on the chunk-major vt production
            for tt in range(8):
                if tt >= 4:
                    t.wait_ge(s_ocp, tt - 3)
                pt = atile[tt % 4]
                for half in range(2):
                    cc = tt * 2 + half
                    t.wait_ge((s_vt, s_vta)[half], tt + 1)
                    for j in range(KT):
                        mm = t.matmul(
                            pt[:, half * 512 : (half + 1) * 512],
                            vt_all[
                                :, (j * NCHK + cc) * VD : (j * NCHK + cc + 1) * VD
                            ],
                            p_sb[:, j * seq + cc * 512 : j * seq + (cc + 1) * 512],
                            start=(j == 0),
                            stop=(j == KT - 1),
                            tile_position=(0, 0),
                        )
                mm.then_inc(s_attn, 1)

        # ---------------- VECTOR ----------------
        @block.vector
        def _(v):
            # The DVE has no intra-engine RAW interlock: any op reading data
            # written by a recent DVE op must sit behind a completion fence
            # (then_inc at writeback + wait_ge). s_vch is the chain counter.
            vch = [0]

            def vfence(inst):
                inst.then_inc(s_vch, 1)
                vch[0] += 1
                v.wait_ge(s_vch, vch[0])

            # k hi/lo split: kT_A = [kh; kh], kT_B = [kl; kl]
            v.wait_ge(s_kproj, 1)
            v.tensor_copy(kT_A[0:64, :], kps[0:64, :])
            vfence(v.tensor_copy(kT_A[64:128, :], kps[64:128, :]))
            v.tensor_tensor(
                kT_B[0:64, :], kps[0:64, :], kT_A[0:64, :], op=ALU.subtract
            )
            v.tensor_tensor(
                kT_B[64:128, :], kps[64:128, :], kT_A[64:128, :], op=ALU.subtract
            ).then_inc(s_ksp, 1)

            # v copy
            v.wait_ge(s_vproj, 1)
            v.tensor_copy(v_sb[:, :], vps[:, :]).then_inc(s_vcp, 1)

            # q splits per section (qU=[ah;bh], qV=[al;bl]) interleaved with
            # the per-chunk maxes to match the tensor's lagged-scores order —
            # split(sec+1) must not sit behind maxes that depend on
            # qproj(sec+2).
            def qsplit(sec):
                # ACT wrote qU = bf16(buf); derive residuals qV = buf - qU
                v.wait_ge(s_qcp, sec + 1)
                buf = (ps_qA, ps_qB)[sec % 2]
                sl = slice(sec * SECW, (sec + 1) * SECW)
                v.tensor_tensor(
                    qV[0:64, sl], buf[0:64, :], qU[0:64, sl], op=ALU.subtract
                )
                v.tensor_tensor(
                    qV[64:128, sl], buf[64:128, :], qU[64:128, sl], op=ALU.subtract
                ).then_inc(s_qsp, 1)

            def maxes(sec):
                for i2 in range(2 * KT):
                    idx = sec * 2 * KT + i2
                    c2, j = divmod(i2, KT)
                    col = j * NCHK + sec * 2 + c2
                    v.wait_ge(s_sc, idx + 1)
                    v.reduce_max(
                        negm[:, col : col + 1], sbank[idx % NBANK], axis=AX,
                        negate=True,
                    ).then_inc(s_mx, 1)

            # maxes(sec) before qsplit(sec+2): splits must never block the
            # max->exp chain the current section's scores rotation waits on
            qsplit(0)
            qsplit(1)
            for sec in range(NSEC):
                maxes(sec)
                if sec + 2 < NSEC:
                    qsplit(sec + 2)

            # combine stats
            v.wait_ge(s_exp, NIDX)
            nv = jv3(negm, KT, NCHK)
            vfence(
                v.tensor_tensor(
                    jv3(mt1, KT, 8), nv[:, :, 0:8], nv[:, :, 8:16], op=ALU.min
                )
            )
            m1 = jv3(mt1, KT, 8)
            vfence(
                v.tensor_tensor(
                    jv3(mt2, KT, 4), m1[:, :, 0:4], m1[:, :, 4:8], op=ALU.min
                )
            )
            m2 = jv3(mt2, KT, 4)
            vfence(
                v.tensor_tensor(
                    jv3(mt3, KT, 2), m2[:, :, 0:2], m2[:, :, 2:4], op=ALU.min
                )
            )
            m3 = jv3(mt3, KT, 2)
            # negM consumed only by ACT (emat bias); then_inc gates it there
            v.tensor_tensor(
                jv3(negM, KT, 1), m3[:, :, 0:1], m3[:, :, 1:2], op=ALU.min
            ).then_inc(s_mM, 1)
            v.wait_ge(s_em, 1)
            vfence(v.tensor_tensor(Sw[:, :], emat[:, :], S_[:, :], op=ALU.mult))
            for j in range(KT):
                ins = v.reduce_sum(
                    Sg[:, j : j + 1], Sw[:, j * NCHK : (j + 1) * NCHK], axis=AX
                )
            vfence(ins)
            v.reciprocal(rS[:, :], Sg[:, :]).then_inc(s_rs, 1)
            v.wait_ge(s_rs, 1)
            for j in range(KT):
                ins = v.tensor_scalar_mul(
                    gg[:, j * NCHK : (j + 1) * NCHK],
                    emat[:, j * NCHK : (j + 1) * NCHK],
                    rS[:, j : j + 1],
                )
            ins.then_inc(s_gq, 1)
            v.wait_ge(s_gq, 1)
            # vt production chunk-major (even chunks; ACT does odd) so attn
            # tile tt can start as soon as chunks 2tt/2tt+1 exist
            for c in range(0, NCHK, 2):
                for j in range(KT):
                    idx = j * NCHK + c
                    ins = v.tensor_scalar_mul(
                        vt_all[:, idx * VD : (idx + 1) * VD],
                        v_sb[:, j * VD : (j + 1) * VD],
                        gg[:, idx : idx + 1],
                    )
                ins.then_inc(s_vt, 1)

            # out copies (single slot: wait for the previous tile's DMA)
            for tt in range(8):
                v.wait_ge(s_attn, tt + 1)
                if tt >= 1:
                    v.wait_ge(s_odp[0], 16 * tt)
                v.tensor_copy(out_sb[:, :], atile[tt % 4]).then_inc(s_ocp, 1)



    nc.finalize()
    return nc


# ------------------------- host side -------------------------

def _split_bf16(x):
    import ml_dtypes

    hi = x.astype(ml_dtypes.bfloat16)
    lo = (x - hi.astype(np.float32)).astype(ml_dtypes.bfloat16)
    return hi, lo


def _tile_cols(xT, w):
    """[d, w] -> [128, (d//128)*w], col dd*w+i = xT[dd*128+p, i]."""
    dd = xT.shape[0] // 128
    return np.ascontiguousarray(
        xT.reshape(dd, 128, w).transpose(1, 0, 2).reshape(128, dd * w)
    )


def build_inputs(inputs):
    """inputs dict -> (nc, in_maps) for the 8 cores."""
    import ml_dtypes

    bf = ml_dtypes.bfloat16
    queries = np.asarray(inputs["queries"], dtype=np.float32)
    keys = np.asarray(inputs["keys"], dtype=np.float32)
    values = np.asarray(inputs["values"], dtype=np.float32)
    seq, d = queries.shape
    ksl = seq // C

    qw8 = (np.asarray(inputs["query_weights"]) / np.sqrt(np.float32(QK))).astype(
        np.float32
    )
    wqh, wql = _split_bf16(qw8)
    wqs = np.concatenate(
        [wqh.astype(np.float32), wql.astype(np.float32)], axis=1
    )  # [d, 128]
    wkh, wkl = _split_bf16(np.asarray(inputs["key_weights"], dtype=np.float32))
    wv = np.asarray(inputs["value_weights"], dtype=np.float32).astype(bf)

    shared = {
        "wqs": _tile_cols(wqs, 2 * QK).astype(bf),
        "wkh": _tile_cols(wkh.astype(np.float32), QK).astype(bf),
        "wkl": _tile_cols(wkl.astype(np.float32), QK).astype(bf),
        "wv": _tile_cols(wv.astype(np.float32), VD).astype(bf),
    }
    qT = np.ascontiguousarray(queries.T)
    qh, ql = _split_bf16(qT)
    NDT, NSEC = d // 128, 8
    SECW = seq // NSEC

    def swizzle(t):
        # [128, dd*seq + i] -> sec-major [128, sec*(NDT*SECW) + dd*SECW + i']
        return np.ascontiguousarray(
            t.reshape(128, NDT, NSEC, SECW)
            .transpose(0, 2, 1, 3)
            .reshape(128, NDT * seq)
        )

    shared["qhT"] = swizzle(_tile_cols(qh.astype(np.float32), seq)).astype(bf)
    shared["qlT"] = swizzle(_tile_cols(ql.astype(np.float32), seq)).astype(bf)

    in_maps = []
    for c in range(C):
        sl = slice(c * ksl, (c + 1) * ksl)
        kT = np.ascontiguousarray(keys[sl].T)
        vTc = np.ascontiguousarray(values[sl].T)
        kh, kl = _split_bf16(kT)
        m = dict(shared)
        m["khT"] = _tile_cols(kh.astype(np.float32), ksl).astype(bf)
        m["klT"] = _tile_cols(kl.astype(np.float32), ksl).astype(bf)
        m["vT"] = _tile_cols(vTc, ksl).astype(bf)
        in_maps.append(m)

    nc = build_nc(seq=seq, d=d)
    return nc, in_maps


def combine_outputs(results):
    acc = np.zeros((VD, SEQ), dtype=np.float32)
    for c in range(C):
        acc += np.asarray(results[c]["out"], dtype=np.float32)
    return np.ascontiguousarray(acc.T)


def run_spmd_staged(nc, in_maps, profile_dir=None):
    """run_bass_via_pjrt with inputs pre-staged on-device (blocks until all
    shards are resident) so the 8 cores launch aligned instead of staggered
    by per-device input-transfer time. Optionally wraps the execute in the
    axon NTFF profile hook (profile_dir)."""
    import jax
    import numpy as np_
    from jax.sharding import Mesh, PartitionSpec, NamedSharding
    from jax.experimental.shard_map import shard_map
    import concourse.mybir as mybir
    from concourse import bass2jax

    bass2jax.install_neuronx_cc_hook()
    n_cores = len(in_maps)

    partition_name = (
        nc.partition_id_tensor.name if nc.partition_id_tensor else None
    )
    in_names, out_names, out_avals, zero_outs = [], [], [], []
    for alloc in nc.m.functions[0].allocations:
        if not isinstance(alloc, mybir.MemoryLocationSet):
            continue
        name = alloc.memorylocations[0].name
        if alloc.kind == "ExternalInput":
            if name != partition_name:
                in_names.append(name)
        elif alloc.kind == "ExternalOutput":
            out_names.append(name)
            shape = tuple(alloc.tensor_shape)
            dtype = mybir.dt.np(alloc.dtype)
            out_avals.append(jax.core.ShapedArray(shape, dtype))
            zero_outs.append(np_.zeros(shape, dtype))
    n_params = len(in_names)
    n_outs = len(out_avals)
    all_names = in_names + out_names
    if partition_name is not None:
        all_names = all_names + [partition_name]

    def _body(*args):
        operands = list(args)
        if partition_name is not None:
            operands.append(bass2jax.partition_id_tensor())
        outs = bass2jax._bass_exec_p.bind(
            *operands,
            out_avals=tuple(out_avals),
            in_names=tuple(all_names),
            out_names=tuple(out_names),
            lowering_input_output_aliases=(),
            sim_require_finite=True,
            sim_require_nnan=True,
            nc=nc,
        )
        return tuple(outs)

    devices = jax.devices()[:n_cores]
    mesh = Mesh(np_.asarray(devices), ("core",))
    spec = NamedSharding(mesh, PartitionSpec("core"))
    sharded = jax.jit(
        shard_map(
            _body,
            mesh=mesh,
            in_specs=(PartitionSpec("core"),) * (n_params + n_outs),
            out_specs=(PartitionSpec("core"),) * n_outs,
            check_rep=False,
        ),
        donate_argnums=tuple(range(n_params, n_params + n_outs)),
        keep_unused=True,
    )
    concat_in = [
        np_.concatenate([np_.asarray(in_maps[c][nm]) for c in range(n_cores)], axis=0)
        for nm in in_names
    ]
    concat_zero = [
        np_.zeros((n_cores * z.shape[0], *z.shape[1:]), z.dtype) for z in zero_outs
    ]
    staged = [jax.device_put(a, spec) for a in concat_in + concat_zero]
    jax.block_until_ready(staged)

    if profile_dir is not None:
        hook = None
        try:
            from antenv.axon_hooks import get_axon_ntff_profile_hook

            hook = get_axon_ntff_profile_hook()
        except ImportError:
            pass
        if hook is None:
            from trn_agent_boot.trn_boot import _ntff_profile_via_ctypes

            hook = _ntff_profile_via_ctypes("/opt/axon/libaxon_pjrt.so")
        with hook(profile_dir, list(range(n_cores))):
            out_arrs = sharded(*staged)
            jax.block_until_ready(out_arrs)
    else:
        out_arrs = sharded(*staged)
    return [
        {
            nm: np_.asarray(out_arrs[i]).reshape(n_cores, *out_avals[i].shape)[c]
            for i, nm in enumerate(out_names)
        }
        for c in range(n_cores)
    ]


def kernel(queries, keys, values, query_weights, key_weights, value_weights):
    import sys

    for p in ("/opt/trn_rl_repo",):
        if p not in sys.path:
            sys.path.insert(0, p)

    nc, in_maps = build_inputs(
        {
            "queries": queries,
            "keys": keys,
            "values": values,
            "query_weights": query_weights,
            "key_weights": key_weights,
            "value_weights": value_weights,
        }
    )
    results = run_spmd_staged(nc, in_maps)
    return combine_outputs(results)


# revision 75
# speedup vs baseline: 1.2102x; 1.2102x over previous
"""Distributed Trainium2 attention-head kernel (softmax over the QUERY axis).

Strategy (8 NeuronCores, SPMD, KEY-dim sharding, zero cross-core comm):
  The softmax normalizes over the query axis (axis 0 of scores). Sharding
  the KEY dim keeps every softmax column fully local to one core: core c
  holds keys/values rows [c*1024:(c+1)*1024] and the FULL queries. It
  computes scoresT[j, i] for its 1024 keys x all 8192 queries, local
  per-key softmax stats, and a full-shape partial output
  outT_c = (attn_slab)^T-contracted with v_slab. The HOST sums the 8
  partials. No device-to-device traffic -> immune to the multi-ms
  execution-start stagger across the 8 PJRT devices (which dominated the
  previous all-to-all design at ~11.7ms).

  Numerics (validated vs reference in fp64/numpy: rel err 2.9e-3):
  - Host pre-splits activations/weights into bf16 hi/lo pairs.
  - 3-term bf16 projections (wh*xh + wh*xl + wl*xh) in fp32 PSUM.
  - Projected q/k re-split hi/lo on device via PSUM bf16-rounding trick.
  - Scores: 2-pass stacked bf16 matmuls: kT_A=[kh;kl], kT_B=[kl;kh]
    against qS=[qh;ql] -> exact (kh+kl)(qh+ql) in fp32 PSUM.
  - Softmax over queries with per-512-chunk max; the exp(m_chunk - m_j)/S_j
    rescale folds into per-chunk vt tiles used as attn@v lhsT.
"""

import numpy as np

C = 8
SEQ = 8192
D = 1024
QK = 64
VD = 64


def build_nc(seq=SEQ, d=D, warmup=True, debug_taps=False):
    import concourse.bacc as bacc
    import concourse.mybir as mybir

    f32 = mybir.dt.float32
    bf16 = mybir.dt.bfloat16
    AX = mybir.AxisListType.X
    ALU = mybir.AluOpType
    ACTF = mybir.ActivationFunctionType

    NDT = d // 128            # 8 d_model tiles
    KSL = seq // C            # 1024 keys per core
    KT = KSL // 128           # 8 key tiles
    NSEC = 8                  # q-projection sections
    SECW = seq // NSEC        # 1024 query cols per section
    NCHK = seq // 512         # 16 score chunks per key tile
    NIDX = KT * NCHK          # 128 (j, c) chunk pairs
    QIN_W = NDT * seq         # 65536 q activation cols
    KIN_W = NDT * KSL         # 8192 k/v activation cols
    PCW = 2048                # q piece width (2 dd per piece, swizzled DRAM)
    NPPS = NDT * SECW * 2 // PCW  # 8 pieces per section (4 per hl)
    NPIECE = NSEC * NPPS      # 64 streamed q pieces

    nc = bacc.Bacc(target_bir_lowering=False, debug=False)

    def din(name, w):
        return nc.declare_dram_parameter(name, [128, w], bf16, isOutput=False)

    qhT_d, qlT_d = din("qhT", QIN_W), din("qlT", QIN_W)
    khT_d, klT_d = din("khT", KIN_W), din("klT", KIN_W)
    vT_d = din("vT", KIN_W)
    wqs_d = din("wqs", NDT * 2 * QK)   # stacked [wqh | wql], 128 cols per dd
    wkh_d, wkl_d = din("wkh", NDT * QK), din("wkl", NDT * QK)
    wv_d = din("wv", NDT * VD)
    out_d = nc.declare_dram_parameter("out", [VD, seq], f32, isOutput=True)
    dbg = {}
    if debug_taps:
        def dout(name, p, w, dt):
            dbg[name] = nc.declare_dram_parameter(name, [p, w], dt, isOutput=True)

        dout("d_qU", 128, seq, bf16)
        dout("d_qV", 128, seq, bf16)
        dout("d_kTA", 128, seq // C, bf16)
        dout("d_kTB", 128, seq // C, bf16)
        dout("d_vsb", 128, (seq // C // 128) * VD, bf16)
        dout("d_negm", 128, (seq // C // 128) * (seq // 512), f32)
        dout("d_S", 128, (seq // C // 128) * (seq // 512), f32)
        dout("d_emat", 128, (seq // C // 128) * (seq // 512), f32)
        dout("d_mt1", 128, (seq // C // 128) * 8, f32)
        dout("d_negM", 128, seq // C // 128, f32)
        dout("d_dmat", 128, (seq // C // 128) * (seq // 512), f32)
        dout("d_Sg", 128, seq // C // 128, f32)
        dout("d_gg", 128, (seq // C // 128) * (seq // 512), f32)
        dout("d_vt", 128, (seq // C // 128) * (seq // 512) * VD, bf16)
        dout("d_P", 128, (seq // C // 128) * seq, bf16)

    from contextlib import ExitStack

    with ExitStack() as ctx:
        block = ctx.enter_context(nc.Block())
        sem = lambda n: ctx.enter_context(nc.semaphore(n))
        sb = lambda n, shape, dt: ctx.enter_context(nc.sbuf_tensor(n, shape, dt))
        ps = lambda n, shape: ctx.enter_context(nc.psum_tensor(n, shape, f32))

        s_in_k = sem("s_in_k")
        s_in_v = sem("s_in_v")
        s_in_q = sem("s_in_q")
        # per-slot DMA-completion sems: a single counting sem is racy when
        # two DMAs are in flight (16 per-engine incs interleave across DMAs)
        NSLOT = 7
        s_qd = [sem(f"s_qd{i}") for i in range(NSLOT)]
        s_odp = [sem("s_od0"), sem("s_od1")]
        s_qcons = sem("s_qcons")
        s_qsp = sem("s_qsp")
        s_kproj = sem("s_kproj")
        s_ksp = sem("s_ksp")
        s_vproj = sem("s_vproj")
        s_vcp = sem("s_vcp")
        s_sc = sem("s_sc")
        s_mx = sem("s_mx")
        s_exp = sem("s_exp")
        s_dm = sem("s_dm")
        s_em = sem("s_em")
        s_mM = sem("s_mM")
        s_rs = sem("s_rs")
        s_gq = sem("s_gq")
        s_vch = sem("s_vch")
        s_vt = sem("s_vt")
        s_vta = sem("s_vta")
        s_qcp = sem("s_qcp")
        s_attn = sem("s_attn")
        s_ocp = sem("s_ocp")
        s_od = sem("s_od")

        # ---- SBUF ----
        p_sb = sb("p_arena", [128, KT * seq], bf16)     # P; aliases k/v acts
        khT = p_sb[:, 0 * KIN_W : 1 * KIN_W]
        klT = p_sb[:, 1 * KIN_W : 2 * KIN_W]
        vT = p_sb[:, 2 * KIN_W : 3 * KIN_W]
        qU = sb("qU", [128, seq], bf16)                 # [ah(0:64); bh(64:128)]
        qV = sb("qV", [128, seq], bf16)                 # [al(0:64); bl(64:128)]
        # streaming slots share storage with vt_all: slots die after the last
        # qproj consumption, vt_all is written at combine (strictly later)
        qvt = sb("qvt_arena", [128, NSLOT * PCW], bf16)
        qslot = qvt[:, :]
        kT_A = sb("kT_A", [128, KSL], bf16)             # [kh; kh]
        kT_B = sb("kT_B", [128, KSL], bf16)             # [kl; kl]
        wqs = sb("wqs_s", [128, NDT * 2 * QK], bf16)
        wkh = sb("wkh_s", [128, NDT * QK], bf16)
        wkl = sb("wkl_s", [128, NDT * QK], bf16)
        wv = sb("wv_s", [128, NDT * VD], bf16)
        v_sb = sb("v_sb", [128, KT * VD], bf16)         # projected v slab
        vt_all = qvt[:, 0 : NIDX * VD]                  # per-(j,c) vt tiles
        negm = sb("negm", [128, NIDX], f32)             # -max per (j,c)
        S_ = sb("S_", [128, NIDX], f32)                 # exp-sum per (j,c)
        mt1 = sb("mt1", [128, KT * 8], f32)
        mt2 = sb("mt2", [128, KT * 4], f32)
        mt3 = sb("mt3", [128, KT * 2], f32)
        negM = sb("negM", [128, KT], f32)               # -max per j
        dmat = sb("dmat", [128, NIDX], f32)             # m_j - m_jc
        emat = sb("emat", [128, NIDX], f32)             # exp(m_jc - m_j)
        Sw = sb("Sw", [128, NIDX], f32)
        Sg = sb("Sg", [128, KT], f32)
        rS = sb("rS", [128, KT], f32)
        gg = sb("gg", [128, NIDX], f32)                 # e/S fold factors
        out_sb = sb("out_sb", [64, 1024], f32)

        # ---- PSUM: 2-bank ping-pong pair for q-proj + 4-bank scratch ----
        ps_qA = ps("ps_qA", [128, SECW])      # 2 banks
        ps_qB = ps("ps_qB", [128, SECW])      # 2 banks
        ps_sc = ps("ps_sc", [128, 2048])      # 4 banks: k/v proj, scores, attn
        kps = ps_sc[:, 0:KSL]                 # k-projection [128, 1024]
        vps = ps_sc[:, KSL : KSL + KT * VD]   # v-projection [128, 512]
        sbank = [
            ps_sc[:, 0:512],
            ps_sc[:, 512:1024],
            ps_sc[:, 1024:1536],
            ps_sc[:, 1536:2048],
        ]
        NBANK = 4
        # all at partition base 0: DVE copies to out_sb must stay lane-aligned
        atile = [
            ps_qA[0:64, :],
            ps_qB[0:64, :],
            ps_sc[0:64, 0:1024],
            ps_sc[0:64, 1024:2048],
        ]

        def jv3(t2d, j, c):
            return t2d[:, 0 : j * c].rearrange("p (j c) -> p j c", j=j, c=c)

        # ---------------- SYNC: all input DMAs + output ----------------
        @block.sync
        def _(s):
            def dma(dst, src, sem_):
                s.dma_start(out=dst, in_=src).then_inc(sem_, 16)

            dma(wkh[:, :], wkh_d[:, :], s_in_k)
            dma(wkl[:, :], wkl_d[:, :], s_in_k)
            dma(khT, khT_d[:, :], s_in_k)
            dma(klT, klT_d[:, :], s_in_k)
            dma(wv[:, :], wv_d[:, :], s_in_v)
            dma(vT, vT_d[:, :], s_in_v)
            dma(wqs[:, :], wqs_d[:, :], s_in_q)
            # q activation stream. DRAM is host-swizzled: col layout is
            # sec-major (sec, dd, i) so a piece = (sec, hl, dd-pair) is one
            # contiguous 512 KB read.
            for p in range(NPIECE):
                sec, rem = divmod(p, NPPS)
                hl, pr = divmod(rem, NPPS // 2)
                if p >= NSLOT:
                    s.wait_ge(s_qcons, p - NSLOT + 1)
                src_d = qhT_d if hl == 0 else qlT_d
                s.dma_start(
                    out=qslot[:, (p % NSLOT) * PCW : (p % NSLOT + 1) * PCW],
                    in_=src_d[:, sec * NDT * SECW + pr * PCW :
                              sec * NDT * SECW + (pr + 1) * PCW],
                ).then_inc(s_qd[p % NSLOT], 16)
            # output tiles (single out_sb slot: one DMA in flight at a time)
            for t in range(8):
                s.wait_ge(s_ocp, t + 1)
                s.dma_start(
                    out=out_d[:, t * 1024 : (t + 1) * 1024],
                    in_=out_sb[:, :],
                ).then_inc(s_odp[0], 16)
            s.wait_ge(s_odp[0], 8 * 16)
            if debug_taps:
                n_dbg = 0
                for name, src in (
                    ("d_qU", qU[:, :]),
                    ("d_qV", qV[:, :]),
                    ("d_kTA", kT_A[:, :]),
                    ("d_kTB", kT_B[:, :]),
                    ("d_vsb", v_sb[:, :]),
                    ("d_negm", negm[:, :]),
                    ("d_S", S_[:, :]),
                    ("d_emat", emat[:, :]),
                    ("d_mt1", mt1[:, :]),
                    ("d_negM", negM[:, :]),
                    ("d_dmat", dmat[:, :]),
                    ("d_Sg", Sg[:, :]),
                    ("d_gg", gg[:, :]),
                    ("d_vt", vt_all[:, :]),
                    ("d_P", p_sb[:, :]),
                ):
                    s.dma_start(out=dbg[name][:, :], in_=src).then_inc(s_odp[0], 16)
                    n_dbg += 1
                s.wait_ge(s_odp[0], (8 + n_dbg) * 16)

        # ---------------- SCALAR (ACT): q-split copies + exp + vt ----------
        @block.scalar
        def _(sc):
            def qcopy(sec):
                # bf16 rounding copies of the projection PSUM; DVE derives
                # the residuals. Runs on ACT so DVE's max->exp chain never
                # waits behind split work.
                sc.wait_ge(s_qcons, NPPS * (sec + 1))
                buf = (ps_qA, ps_qB)[sec % 2]
                sl = slice(sec * SECW, (sec + 1) * SECW)
                sc.activation(qU[0:64, sl], buf[0:64, :], ACTF.Copy)
                sc.activation(qU[64:128, sl], buf[64:128, :], ACTF.Copy)\
                    .then_inc(s_qcp, 1)

            def exps(sec, lo, hi):
                for i2 in range(lo, hi):
                    idx = sec * 2 * KT + i2
                    c2, j = divmod(i2, KT)
                    c = sec * 2 + c2
                    col = j * NCHK + c
                    sc.wait_ge(s_mx, idx + 1)
                    sc.activation(
                        p_sb[:, j * seq + c * 512 : j * seq + (c + 1) * 512],
                        sbank[idx % NBANK],
                        ACTF.Exp,
                        bias=negm[:, col : col + 1],
                        scale=1.0,
                        accum_out=S_[:, col : col + 1],
                    ).then_inc(s_exp, 1)

            # ACT does only section 0's qcopy (before any exps are pending);
            # later sections' splits run on the DVE inside the maxes loop
            # where their waits are already satisfied.
            qcopy(0)
            for sec in range(NSEC):
                exps(sec, 0, 2 * KT)

            # combine: emat_j = exp(-negm_jc + negM_j) = exp(m_jc - m_j)
            sc.wait_ge(s_mM, 1)
            for j in range(KT):
                ins = sc.activation(
                    emat[:, j * NCHK : (j + 1) * NCHK],
                    negm[:, j * NCHK : (j + 1) * NCHK],
                    ACTF.Exp,
                    scale=-1.0,
                    bias=negM[:, j : j + 1],
                )
            ins.then_inc(s_em, 1)

            # odd-chunk vt tiles: vt = v * gg via Copy-with-scale
            sc.wait_ge(s_gq, 1)
            for c in range(1, NCHK, 2):
                for j in range(KT):
                    idx = j * NCHK + c
                    ins = sc.activation(
                        vt_all[:, idx * VD : (idx + 1) * VD],
                        v_sb[:, j * VD : (j + 1) * VD],
                        ACTF.Copy,
                        scale=gg[:, idx : idx + 1],
                    )
                ins.then_inc(s_vta, 1)

        # ---------------- TENSOR ----------------
        @block.tensor
        def _(t):
            # HAM warm-up junk matmuls while k/v DMAs stream.
            for w in range(58 if warmup else 0):
                t.matmul(
                    kps[0:64, 0:512],
                    p_sb[:, 0:64],
                    p_sb[:, 64 : 64 + 512],
                    start=(w == 0),
                    stop=False,
                )

            # k projection -> kps, duplicated on both partition halves.
            t.wait_ge(s_in_k, 64)
            for pos in (0, 64):
                for dd in range(NDT):
                    for ti, (W, X) in enumerate(
                        ((wkh, khT), (wkh, klT), (wkl, khT))
                    ):
                        for n in range(2):
                            mm = t.matmul(
                                kps[pos : pos + 64, n * 512 : (n + 1) * 512],
                                W[:, dd * QK : (dd + 1) * QK],
                                X[:, dd * KSL + n * 512 : dd * KSL + (n + 1) * 512],
                                start=(dd == 0 and ti == 0),
                                stop=(dd == NDT - 1 and ti == 2),
                                tile_position=(0, pos),
                            )
            mm.then_inc(s_kproj, 1)

            # v projection -> vps
            t.wait_ge(s_in_v, 32)
            for tau in range(KT):
                for dd in range(NDT):
                    mm = t.matmul(
                        vps[:, tau * VD : (tau + 1) * VD],
                        vT[:, dd * KSL + tau * 128 : dd * KSL + tau * 128 + 128],
                        wv[:, dd * VD : (dd + 1) * VD],
                        start=(dd == 0),
                        stop=(dd == NDT - 1),
                    )
            mm.then_inc(s_vproj, 1)

            # q projection in 8 sections of 1024 (stacked [wqh|wql] weights:
            # psum[0:64]=a, psum[64:128]=b), scores lag one section behind
            t.wait_ge(s_in_q, 16)

            def qproj(sec):
                if sec >= 2:
                    t.wait_ge(s_qsp, sec - 1)
                buf = (ps_qA, ps_qB)[sec % 2]
                for hl in range(2):
                    for dd in range(NDT):
                        pp = sec * NPPS + hl * (NPPS // 2) + dd // 2
                        half = (dd % 2) * SECW
                        if dd % 2 == 0:
                            t.wait_ge(s_qd[pp % NSLOT], 16 * (pp // NSLOT + 1))
                        X = qslot[:, (pp % NSLOT) * PCW + half :
                                  (pp % NSLOT) * PCW + half + SECW]
                        for n in range(2):
                            mm = t.matmul(
                                buf[:, n * 512 : (n + 1) * 512],
                                wqs[:, dd * 128 : (dd + 1) * 128],
                                X[:, n * 512 : (n + 1) * 512],
                                start=(hl == 0 and dd == 0),
                                stop=(hl == 1 and dd == NDT - 1),
                            )
                        if dd % 2 == 1:
                            mm.then_inc(s_qcons, 1)

            def scores(sec):
                t.wait_ge(s_qsp, sec + 1)
                for c2 in range(2):
                    c = sec * 2 + c2
                    for j in range(KT):
                        idx = sec * 2 * KT + c2 * KT + j
                        if idx >= NBANK:
                            t.wait_ge(s_exp, idx - NBANK + 1)
                        bank = sbank[idx % NBANK]
                        t.matmul(
                            bank,
                            kT_A[:, j * 128 : (j + 1) * 128],
                            qU[:, c * 512 : (c + 1) * 512],
                            start=True,
                            stop=False,
                        )
                        t.matmul(
                            bank,
                            kT_A[:, j * 128 : (j + 1) * 128],
                            qV[:, c * 512 : (c + 1) * 512],
                            start=False,
                            stop=False,
                        )
                        mm = t.matmul(
                            bank,
                            kT_B[:, j * 128 : (j + 1) * 128],
                            qU[:, c * 512 : (c + 1) * 512],
                            start=False,
                            stop=True,
                        )
                        mm.then_inc(s_sc, 1)

            for sec in range(NSEC):
                qproj(sec)
                if sec == 0:
                    # scores write ps_sc: k/v proj results must be consumed
                    t.wait_ge(s_ksp, 1)
                    t.wait_ge(s_vcp, 1)
                else:
                    scores(sec - 1)
            scores(NSEC - 1)

            # attn@v: 8 output tiles of [64, 1024], accumulate over j;
            # per-tile gating on the chunk-major vt production
            for tt in range(8):
                if tt >= 4:
                    t.wait_ge(s_ocp, tt - 3)
                pt = atile[tt % 4]
                for half in range(2):
                    cc = tt * 2 + half
                    t.wait_ge((s_vt, s_vta)[half], tt + 1)
                    for j in range(KT):
                        mm = t.matmul(
                            pt[:, half * 512 : (half + 1) * 512],
                            vt_all[
                                :, (j * NCHK + cc) * VD : (j * NCHK + cc + 1) * VD
                            ],
                            p_sb[:, j * seq + cc * 512 : j * seq + (cc + 1) * 512],
                            start=(j == 0),
                            stop=(j == KT - 1),
                            tile_position=(0, 0),
                        )
                mm.then_inc(s_attn, 1)

        # ---------------- VECTOR ----------------
        @block.vector
        def _(v):
            # The DVE has no intra-engine RAW interlock: any op reading data
            # written by a recent DVE op must sit behind a completion fence
            # (then_inc at writeback + wait_ge). s_vch is the chain counter.
            vch = [0]

            def vfence(inst):
                inst.then_inc(s_vch, 1)
                vch[0] += 1
                v.wait_ge(s_vch, vch[0])

            # k hi/lo split: kT_A = [kh; kh], kT_B = [kl; kl]
            v.wait_ge(s_kproj, 1)
            v.tensor_copy(kT_A[0:64, :], kps[0:64, :])
            vfence(v.tensor_copy(kT_A[64:128, :], kps[64:128, :]))
            v.tensor_tensor(
                kT_B[0:64, :], kps[0:64, :], kT_A[0:64, :], op=ALU.subtract
            )
            v.tensor_tensor(
                kT_B[64:128, :], kps[64:128, :], kT_A[64:128, :], op=ALU.subtract
            ).then_inc(s_ksp, 1)

            # v copy
            v.wait_ge(s_vproj, 1)
            v.tensor_copy(v_sb[:, :], vps[:, :]).then_inc(s_vcp, 1)

            # q splits per section (qU=[ah;bh], qV=[al;bl]) interleaved with
            # the per-chunk maxes to match the tensor's lagged-scores order —
            # split(sec+1) must not sit behind maxes that depend on
            # qproj(sec+2).
            pend = {}

            def qcopies_dve(s):
                # bf16 rounding copies of section s's projection PSUM;
                # gated wait is pre-satisfied at the call site
                v.wait_ge(s_qcons, NPPS * (s + 1))
                buf = (ps_qA, ps_qB)[s % 2]
                sl = slice(s * SECW, (s + 1) * SECW)
                v.tensor_copy(qU[0:64, sl], buf[0:64, :])
                ins = v.tensor_copy(qU[64:128, sl], buf[64:128, :])
                ins.then_inc(s_vch, 1)
                vch[0] += 1
                pend[s] = vch[0]

            def qsubs(s):
                # residuals qV = buf - qU (qU by ACT for s=0, DVE otherwise)
                if s == 0:
                    v.wait_ge(s_qcp, 1)
                else:
                    v.wait_ge(s_vch, pend[s])
                buf = (ps_qA, ps_qB)[s % 2]
                sl = slice(s * SECW, (s + 1) * SECW)
                v.tensor_tensor(
                    qV[0:64, sl], buf[0:64, :], qU[0:64, sl], op=ALU.subtract
                )
                v.tensor_tensor(
                    qV[64:128, sl], buf[64:128, :], qU[64:128, sl], op=ALU.subtract
                ).then_inc(s_qsp, 1)

            # split(sec+1) embedded in maxes(sec): by the time max[1] of
            # scores(sec) has run, qproj(sec+1) is complete, so the copy's
            # wait passes instantly and exps are never starved
            qsubs(0)
            for sec in range(NSEC):
                for i2 in range(2 * KT):
                    idx = sec * 2 * KT + i2
                    c2, j = divmod(i2, KT)
                    col = j * NCHK + sec * 2 + c2
                    v.wait_ge(s_sc, idx + 1)
                    v.reduce_max(
                        negm[:, col : col + 1], sbank[idx % NBANK], axis=AX,
                        negate=True,
                    ).then_inc(s_mx, 1)
                    if i2 == 1 and sec + 1 < NSEC:
                        qcopies_dve(sec + 1)
                if sec + 1 < NSEC:
                    qsubs(sec + 1)

            # combine stats
            v.wait_ge(s_exp, NIDX)
            nv = jv3(negm, KT, NCHK)
            vfence(
                v.tensor_tensor(
                    jv3(mt1, KT, 8), nv[:, :, 0:8], nv[:, :, 8:16], op=ALU.min
                )
            )
            m1 = jv3(mt1, KT, 8)
            vfence(
                v.tensor_tensor(
                    jv3(mt2, KT, 4), m1[:, :, 0:4], m1[:, :, 4:8], op=ALU.min
                )
            )
            m2 = jv3(mt2, KT, 4)
            vfence(
                v.tensor_tensor(
                    jv3(mt3, KT, 2), m2[:, :, 0:2], m2[:, :, 2:4], op=ALU.min
                )
            )
            m3 = jv3(mt3, KT, 2)
            # negM consumed only by ACT (emat bias); then_inc gates it there
            v.tensor_tensor(
                jv3(negM, KT, 1), m3[:, :, 0:1], m3[:, :, 1:2], op=ALU.min
            ).then_inc(s_mM, 1)
            v.wait_ge(s_em, 1)
            vfence(v.tensor_tensor(Sw[:, :], emat[:, :], S_[:, :], op=ALU.mult))
            for j in range(KT):
                ins = v.reduce_sum(
                    Sg[:, j : j + 1], Sw[:, j * NCHK : (j + 1) * NCHK], axis=AX
                )
            vfence(ins)
            v.reciprocal(rS[:, :], Sg[:, :]).then_inc(s_rs, 1)
            v.wait_ge(s_rs, 1)
            for j in range(KT):
                ins = v.tensor_scalar_mul(
                    gg[:, j * NCHK : (j + 1) * NCHK],
                    emat[:, j * NCHK : (j + 1) * NCHK],
                    rS[:, j : j + 1],
                )
            ins.then_inc(s_gq, 1)
            v.wait_ge(s_gq, 1)
            # vt production chunk-major (even chunks; ACT does odd) so attn
            # tile tt can start as soon as chunks 2tt/2tt+1 exist
            for c in range(0, NCHK, 2):
                for j in range(KT):
                    idx = j * NCHK + c
                    ins = v.tensor_scalar_mul(
                        vt_all[:, idx * VD : (idx + 1) * VD],
                        v_sb[:, j * VD : (j + 1) * VD],
                        gg[:, idx : idx + 1],
                    )
                ins.then_inc(s_vt, 1)

            # out copies (single slot: wait for the previous tile's DMA)
            for tt in range(8):
                v.wait_ge(s_attn, tt + 1)
                if tt >= 1:
                    v.wait_ge(s_odp[0], 16 * tt)
                v.tensor_copy(out_sb[:, :], atile[tt % 4]).then_inc(s_ocp, 1)



    nc.finalize()
    return nc


# ------------------------- host side -------------------------

def _split_bf16(x):
    import ml_dtypes

    hi = x.astype(ml_dtypes.bfloat16)
    lo = (x - hi.astype(np.float32)).astype(ml_dtypes.bfloat16)
    return hi, lo


def _tile_cols(xT, w):
    """[d, w] -> [128, (d//128)*w], col dd*w+i = xT[dd*128+p, i]."""
    dd = xT.shape[0] // 128
    return np.ascontiguousarray(
        xT.reshape(dd, 128, w).transpose(1, 0, 2).reshape(128, dd * w)
    )


def build_inputs(inputs):
    """inputs dict -> (nc, in_maps) for the 8 cores."""
    import ml_dtypes

    bf = ml_dtypes.bfloat16
    queries = np.asarray(inputs["queries"], dtype=np.float32)
    keys = np.asarray(inputs["keys"], dtype=np.float32)
    values = np.asarray(inputs["values"], dtype=np.float32)
    seq, d = queries.shape
    ksl = seq // C

    qw8 = (np.asarray(inputs["query_weights"]) / np.sqrt(np.float32(QK))).astype(
        np.float32
    )
    wqh, wql = _split_bf16(qw8)
    wqs = np.concatenate(
        [wqh.astype(np.float32), wql.astype(np.float32)], axis=1
    )  # [d, 128]
    wkh, wkl = _split_bf16(np.asarray(inputs["key_weights"], dtype=np.float32))
    wv = np.asarray(inputs["value_weights"], dtype=np.float32).astype(bf)

    shared = {
        "wqs": _tile_cols(wqs, 2 * QK).astype(bf),
        "wkh": _tile_cols(wkh.astype(np.float32), QK).astype(bf),
        "wkl": _tile_cols(wkl.astype(np.float32), QK).astype(bf),
        "wv": _tile_cols(wv.astype(np.float32), VD).astype(bf),
    }
    qT = np.ascontiguousarray(queries.T)
    qh, ql = _split_bf16(qT)
    NDT, NSEC = d // 128, 8
    SECW = seq // NSEC

    def swizzle(t):
        # [128, dd*seq + i] -> sec-major [128, sec*(NDT*SECW) + dd*SECW + i']
        return np.ascontiguousarray(
            t.reshape(128, NDT, NSEC, SECW)
            .transpose(0, 2, 1, 3)
            .reshape(128, NDT * seq)
        )

    shared["qhT"] = swizzle(_tile_cols(qh.astype(np.float32), seq)).astype(bf)
    shared["qlT"] = swizzle(_tile_cols(ql.astype(np.float32), seq)).astype(bf)

    in_maps = []
    for c in range(C):
        sl = slice(c * ksl, (c + 1) * ksl)
        kT = np.ascontiguousarray(keys[sl].T)
        vTc = np.ascontiguousarray(values[sl].T)
        kh, kl = _split_bf16(kT)
        m = dict(shared)
        m["khT"] = _tile_cols(kh.astype(np.float32), ksl).astype(bf)
        m["klT"] = _tile_cols(kl.astype(np.float32), ksl).astype(bf)
        m["vT"] = _tile_cols(vTc, ksl).astype(bf)
        in_maps.append(m)

    nc = build_nc(seq=seq, d=d)
    return nc, in_maps


def combine_outputs(results):
    acc = np.zeros((VD, SEQ), dtype=np.float32)
    for c in range(C):
        acc += np.asarray(results[c]["out"], dtype=np.float32)
    return np.ascontiguousarray(acc.T)


def run_spmd_staged(nc, in_maps, profile_dir=None):
    """run_bass_via_pjrt with inputs pre-staged on-device (blocks until all
    shards are resident) so the 8 cores launch aligned instead of staggered
    by per-device input-transfer time. Optionally wraps the execute in the
    axon NTFF profile hook (profile_dir)."""
    import jax
    import numpy as np_
    from jax.sharding import Mesh, PartitionSpec, NamedSharding
    from jax.experimental.shard_map import shard_map
    import concourse.mybir as mybir
    from concourse import bass2jax

    bass2jax.install_neuronx_cc_hook()
    n_cores = len(in_maps)

    partition_name = (
        nc.partition_id_tensor.name if nc.partition_id_tensor else None
    )
    in_names, out_names, out_avals, zero_outs = [], [], [], []
    for alloc in nc.m.functions[0].allocations:
        if not isinstance(alloc, mybir.MemoryLocationSet):
            continue
        name = alloc.memorylocations[0].name
        if alloc.kind == "ExternalInput":
            if name != partition_name:
                in_names.append(name)
        elif alloc.kind == "ExternalOutput":
            out_names.append(name)
            shape = tuple(alloc.tensor_shape)
            dtype = mybir.dt.np(alloc.dtype)
            out_avals.append(jax.core.ShapedArray(shape, dtype))
            zero_outs.append(np_.zeros(shape, dtype))
    n_params = len(in_names)
    n_outs = len(out_avals)
    all_names = in_names + out_names
    if partition_name is not None:
        all_names = all_names + [partition_name]

    def _body(*args):
        operands = list(args)
        if partition_name is not None:
            operands.append(bass2jax.partition_id_tensor())
        outs = bass2jax._bass_exec_p.bind(
            *operands,
            out_avals=tuple(out_avals),
            in_names=tuple(all_names),
            out_names=tuple(out_names),
            lowering_input_output_aliases=(),
            sim_require_finite=True,
            sim_require_nnan=True,
            nc=nc,
        )
        return tuple(outs)

    devices = jax.devices()[:n_cores]
    mesh = Mesh(np_.asarray(devices), ("core",))
    spec = NamedSharding(mesh, PartitionSpec("core"))
    sharded = jax.jit(
        shard_map(
            _body,
            mesh=mesh,
            in_specs=(PartitionSpec("core"),) * (n_params + n_outs),
            out_specs=(PartitionSpec("core"),) * n_outs,
            check_rep=False,
        ),
        donate_argnums=tuple(range(n_params, n_params + n_outs)),
        keep_unused=True,
    )
    concat_in = [
        np_.concatenate([np_.asarray(in_maps[c][nm]) for c in range(n_cores)], axis=0)
        for nm in in_names
    ]
    concat_zero = [
        np_.zeros((n_cores * z.shape[0], *z.shape[1:]), z.dtype) for z in zero_outs
    ]
    staged = [jax.device_put(a, spec) for a in concat_in + concat_zero]
    jax.block_until_ready(staged)

    if profile_dir is not None:
        hook = None
        try:
            from antenv.axon_hooks import get_axon_ntff_profile_hook

            hook = get_axon_ntff_profile_hook()
        except ImportError:
            pass
        if hook is None:
            from trn_agent_boot.trn_boot import _ntff_profile_via_ctypes

            hook = _ntff_profile_via_ctypes("/opt/axon/libaxon_pjrt.so")
        with hook(profile_dir, list(range(n_cores))):
            out_arrs = sharded(*staged)
            jax.block_until_ready(out_arrs)
    else:
        out_arrs = sharded(*staged)
    return [
        {
            nm: np_.asarray(out_arrs[i]).reshape(n_cores, *out_avals[i].shape)[c]
            for i, nm in enumerate(out_names)
        }
        for c in range(n_cores)
    ]


def kernel(queries, keys, values, query_weights, key_weights, value_weights):
    import sys

    for p in ("/opt/trn_rl_repo",):
        if p not in sys.path:
            sys.path.insert(0, p)

    nc, in_maps = build_inputs(
        {
            "queries": queries,
            "keys": keys,
            "values": values,
            "query_weights": query_weights,
            "key_weights": key_weights,
            "value_weights": value_weights,
        }
    )
    results = run_spmd_staged(nc, in_maps)
    return combine_outputs(results)
